# revision 7
# baseline (speedup 1.0000x reference)
"""Trainium2 Bass kernel for the LoRA-update contraction.

Computes out[b,n] = sum_l <B_l @ A_l, gradient[l,b,n]>_F for
  lora_A    [48, 8, 1024]       (L, R, IN)
  lora_B    [48, 1024, 8]       (L, OUT, R)
  gradient  [48, 4, 2, 1024, 1024]  (L, B, N, OUT, IN)

Strategy (memory-bound problem — gradient is 1.6 GB):
  - Shard L across the 8 NeuronCores (6 layers each, ~201 MB of gradient per
    core). Per-core partial outputs are summed on the host.
  - Per core: W_l = B_l @ A_l is computed on the TensorEngine from bf16
    copies of A/B, one layer ahead of the gradient stream so layer
    boundaries never stall the DMA ring.  W is kept in bf16 (the output
    tolerates ~0.5% error; measured ~2e-3 against fp32 reference).
  - The gradient is stored partition-major ([128, 8192] per (l,b,n) slab,
    rows 8p..8p+7 on partition p) so each 4 MB slab is one DMA of 128
    contiguous 32 KB HBM reads.  A fused VectorEngine scalar_tensor_tensor
    accumulates acc[p, l*8+j] = sum_f G[p,f]*W[p,f] per slab in one pass.
  - B's columns are pre-permuted on the host so the W matmuls produce W
    directly in the same partition-major layout.
  - The 128-partition accumulators are reduced on-chip with a ones-vector
    matmul, so the kernel ends with a single tiny HBM write instead of a
    slow 128-descriptor strided store.
"""

import numpy as np

L, R, OUT, IN = 48, 8, 1024, 1024
B, N = 4, 2
NCORES = 8
LP = L // NCORES  # layers per core
BN = B * N

_PART = 128
_RPP = OUT // _PART          # gradient rows per partition (8)
_FREE = _RPP * IN            # free dim of one (l,j) slab (8192)
_TAIL1 = 7168                # first tail piece of the final slab
_NCELL = LP * BN + 1         # acc cells: one per slab + final tail piece


def build_module(lp=LP, bn=BN, in_dim=IN, r=R):
    """Build + compile the per-core Bass module (same program on all cores)."""
    import concourse.bacc as bacc
    import concourse.mybir as mybir
    from concourse.tile import TileContext

    fp32 = mybir.dt.float32
    bf16 = mybir.dt.bfloat16
    n_mm = 512                     # matmul moving-dim tile (one PSUM bank)
    ih = in_dim // n_mm

    nc = bacc.Bacc("TRN2", target_bir_lowering=False, debug=False)

    g = nc.dram_tensor("g", [lp, bn, _PART, _FREE], fp32, kind="ExternalInput").ap()
    # ab[r, l*2048 + 0:1024] = B^T with columns permuted so matmul chunk c
    # yields W rows 8p+c on partition p; ab[r, l*2048 + 1024:2048] = A[l, r].
    ab = nc.dram_tensor("ab", [r, lp * 2 * in_dim], bf16, kind="ExternalInput").ap()
    # One partial sum per slab (plus the final tail piece), already reduced
    # over partitions; host does the tiny (l, j) regrouping.
    out = nc.dram_tensor("out", [_NCELL, 1], fp32, kind="ExternalOutput").ap()

    with TileContext(nc) as tc:
        with (
            tc.tile_pool(name="gpool", bufs=3) as gpool,
            tc.tile_pool(name="wpool", bufs=2) as wpool,
            tc.tile_pool(name="spool", bufs=1) as spool,
            tc.tile_pool(name="small", bufs=1) as small,
            tc.tile_pool(name="pspool", bufs=4, space="PSUM") as pspool,
            tc.tile_pool(name="psf", bufs=1, space="PSUM") as psf,
        ):
            ab_t = small.tile([r, lp * 2 * in_dim], bf16)
            nc.scalar.dma_start(out=ab_t[:], in_=ab)
            acc = small.tile([_PART, _NCELL], fp32)
            ones = small.tile([_PART, 1], fp32)
            nc.gpsimd.memset(ones[:], 1.0)

            def build_w(l):
                # W_l[8p+c, i] on partition p at w[:, c*1024 + i]
                w = wpool.tile([_PART, _FREE], bf16, tag="w")
                base = l * 2 * in_dim
                for c in range(_RPP):
                    for h in range(ih):
                        ps = pspool.tile([_PART, n_mm], fp32, tag="ps")
                        nc.tensor.matmul(
                            ps[:],
                            lhsT=ab_t[:, base + c * _PART:base + (c + 1) * _PART],
                            rhs=ab_t[:, base + in_dim + h * n_mm:
                                     base + in_dim + (h + 1) * n_mm],
                            start=True,
                            stop=True,
                        )
                        nc.scalar.copy(
                            out=w[:, c * in_dim + h * n_mm:
                                  c * in_dim + (h + 1) * n_mm],
                            in_=ps[:],
                        )
                return w

            def stt(gt, wsl, cell):
                sc = spool.tile([_PART, gt.shape[1]], bf16, tag="sc")
                nc.vector.scalar_tensor_tensor(
                    out=sc[:],
                    in0=gt[:],
                    scalar=1.0,
                    in1=wsl,
                    op0=mybir.AluOpType.mult,
                    op1=mybir.AluOpType.mult,
                    accum_out=acc[:, cell:cell + 1],
                )

            ws = [build_w(0), build_w(1)]

            for l in range(lp):
                w = ws[l]
                for j in range(bn):
                    cell = l * bn + j
                    if l == lp - 1 and j == bn - 1:
                        # Final slab in two pieces so the very last STT (and
                        # the closing reduction) starts on a small transfer.
                        gt = gpool.tile([_PART, _TAIL1], fp32, tag="g")
                        nc.sync.dma_start(out=gt[:], in_=g[l, j][:, :_TAIL1])
                        stt(gt, w[:, :_TAIL1], cell)
                        gt2 = gpool.tile([_PART, _FREE - _TAIL1], fp32, tag="g")
                        nc.sync.dma_start(out=gt2[:], in_=g[l, j][:, _TAIL1:])
                        stt(gt2, w[:, _TAIL1:], _NCELL - 1)
                    else:
                        gt = gpool.tile([_PART, _FREE], fp32, tag="g")
                        nc.sync.dma_start(out=gt[:], in_=g[l, j])
                        stt(gt, w[:], cell)
                if l + 2 < lp:
                    ws.append(build_w(l + 2))

            # Reduce acc over the 128 partitions on-chip: ps[m] = sum_p acc[p, m]
            fps = psf.tile([_NCELL, 1], fp32)
            nc.tensor.matmul(fps[:], lhsT=acc[:], rhs=ones[:], start=True, stop=True)
            ft = small.tile([_NCELL, 1], fp32)
            nc.scalar.copy(out=ft[:], in_=fps[:])
            nc.sync.dma_start(out=out, in_=ft[:])

    nc.compile()
    return nc


_NC_CACHE = {}


def _get_module():
    if "nc" not in _NC_CACHE:
        _NC_CACHE["nc"] = build_module()
    return _NC_CACHE["nc"]


def make_in_maps(lora_A, lora_B, gradient):
    import ml_dtypes

    lora_A = np.asarray(lora_A, dtype=np.float32)
    lora_B = np.asarray(lora_B, dtype=np.float32)
    gradient = np.asarray(gradient, dtype=np.float32)
    in_maps = []
    for c in range(NCORES):
        sl = slice(LP * c, LP * (c + 1))
        # btp[l, r, c*128+p] = B[l, 8p+c, r]
        bt = lora_B[sl].transpose(0, 2, 1)
        btp = bt.reshape(LP, R, _PART, _RPP).transpose(0, 1, 3, 2).reshape(
            LP, R, OUT)
        ab = np.concatenate([btp, lora_A[sl]], axis=2)  # [LP, R, 2048]
        ab = ab.transpose(1, 0, 2).reshape(R, LP * 2 * IN)
        in_maps.append({
            "g": np.ascontiguousarray(
                gradient[sl].reshape(LP, BN, _PART, _FREE)),
            "ab": np.ascontiguousarray(ab.astype(ml_dtypes.bfloat16)),
        })
    return in_maps


def kernel(lora_A, lora_B, gradient, _trace=False, _trace_kwargs=None):
    from concourse.bass_utils import run_bass_kernel_spmd

    nc = _get_module()
    in_maps = make_in_maps(lora_A, lora_B, gradient)
    last_exc = None
    for attempt in range(3):
        try:
            res = run_bass_kernel_spmd(
                nc,
                in_maps,
                core_ids=list(range(NCORES)),
                trace=_trace,
                **(_trace_kwargs or {}),
            )
            break
        except Exception as e:  # transient device wedges (NRT_EXEC_UNIT_...)
            last_exc = e
            import time as _time

            _time.sleep(15 * (attempt + 1))
    else:
        raise last_exc
    total = np.zeros(BN, np.float64)
    for m in res.results:
        cells = m["out"].astype(np.float64).ravel()  # [LP*BN + 1]
        per_j = cells[:LP * BN].reshape(LP, BN).sum(axis=0)
        per_j[BN - 1] += cells[LP * BN]
        total += per_j
    out = total.astype(np.float32).reshape(B, N)
    if _trace:
        return out, res
    return out


# revision 12
# speedup vs baseline: 1.0504x; 1.0504x over previous
"""Trainium2 Bass kernel for the LoRA-update contraction.

Computes out[b,n] = sum_l <B_l @ A_l, gradient[l,b,n]>_F for
  lora_A    [48, 8, 1024]       (L, R, IN)
  lora_B    [48, 1024, 8]       (L, OUT, R)
  gradient  [48, 4, 2, 1024, 1024]  (L, B, N, OUT, IN)

Strategy (memory-bound problem — gradient is 1.6 GB):
  - Shard L across the 8 NeuronCores (6 layers each, ~201 MB of gradient per
    core). Per-core partial outputs are summed on the host.
  - Per core: W_l = B_l @ A_l is computed on the TensorEngine from bf16
    copies of A/B, one layer ahead of the gradient stream so layer
    boundaries never stall the DMA ring.  W is kept in bf16 (the output
    tolerates ~0.5% error; measured ~2e-3 against fp32 reference).
  - The gradient is stored partition-major ([128, 8192] per (l,b,n) slab,
    rows 8p..8p+7 on partition p) so each 4 MB slab is one DMA of 128
    contiguous 32 KB HBM reads.  A fused VectorEngine scalar_tensor_tensor
    accumulates acc[p, l*8+j] = sum_f G[p,f]*W[p,f] per slab in one pass.
  - B's columns are pre-permuted on the host so the W matmuls produce W
    directly in the same partition-major layout.
  - The 128-partition accumulators are reduced on-chip with a ones-vector
    matmul, so the kernel ends with a single tiny HBM write instead of a
    slow 128-descriptor strided store.
"""

import numpy as np

L, R, OUT, IN = 48, 8, 1024, 1024
B, N = 4, 2
NCORES = 8
LP = L // NCORES  # layers per core
BN = B * N

_PART = 128
_RPP = OUT // _PART          # gradient rows per partition (8)
_FREE = _RPP * IN            # free dim of one (l,j) slab (8192)
_TAIL1 = 7168                # first tail piece of the final slab
_NCELL = LP * BN + 1         # acc cells: one per slab + final tail piece


def build_module(lp=LP, bn=BN, in_dim=IN, r=R):
    """Build + compile the per-core Bass module (same program on all cores)."""
    import concourse.bacc as bacc
    import concourse.mybir as mybir
    from concourse.tile import TileContext

    fp32 = mybir.dt.float32
    bf16 = mybir.dt.bfloat16
    n_mm = 512                     # matmul moving-dim tile (one PSUM bank)
    ih = in_dim // n_mm

    nc = bacc.Bacc("TRN2", target_bir_lowering=False, debug=False)

    g = nc.dram_tensor("g", [lp, bn, _PART, _FREE], fp32, kind="ExternalInput").ap()
    # ab[l, r, 0:1024] = B^T with columns permuted so matmul chunk c
    # yields W rows 8p+c on partition p; ab[l, r, 1024:2048] = A[l, r].
    ab = nc.dram_tensor("ab", [lp, r, 2 * in_dim], bf16, kind="ExternalInput").ap()
    # One partial sum per slab (plus the final tail piece), already reduced
    # over partitions; host does the tiny (l, j) regrouping.
    out = nc.dram_tensor("out", [_NCELL, 1], fp32, kind="ExternalOutput").ap()

    with TileContext(nc) as tc:
        with (
            tc.tile_pool(name="gpool", bufs=4) as gpool,
            tc.tile_pool(name="wpool", bufs=2) as wpool,
            tc.tile_pool(name="abpool", bufs=3) as abpool,
            tc.tile_pool(name="spool", bufs=1) as spool,
            tc.tile_pool(name="small", bufs=1) as small,
            tc.tile_pool(name="pspool", bufs=4, space="PSUM") as pspool,
            tc.tile_pool(name="psf", bufs=1, space="PSUM") as psf,
        ):
            acc = small.tile([_PART, _NCELL], fp32)
            ones = small.tile([_PART, 1], fp32)
            nc.gpsimd.memset(ones[:], 1.0)

            def load_ab(l):
                ab_t = abpool.tile([r, 2 * in_dim], bf16, tag="ab")
                nc.scalar.dma_start(out=ab_t[:], in_=ab[l])
                return ab_t

            def build_w(ab_t):
                # W_l[8p+c, i] on partition p at w[:, c*1024 + i]
                w = wpool.tile([_PART, _FREE], bf16, tag="w")
                for c in range(_RPP):
                    for h in range(ih):
                        ps = pspool.tile([_PART, n_mm], fp32, tag="ps")
                        nc.tensor.matmul(
                            ps[:],
                            lhsT=ab_t[:, c * _PART:(c + 1) * _PART],
                            rhs=ab_t[:, in_dim + h * n_mm:
                                     in_dim + (h + 1) * n_mm],
                            start=True,
                            stop=True,
                        )
                        nc.scalar.copy(
                            out=w[:, c * in_dim + h * n_mm:
                                  c * in_dim + (h + 1) * n_mm],
                            in_=ps[:],
                        )
                return w

            def stt(gt, wsl, cell):
                sc = spool.tile([_PART, gt.shape[1]], bf16, tag="sc")
                nc.vector.scalar_tensor_tensor(
                    out=sc[:],
                    in0=gt[:],
                    scalar=1.0,
                    in1=wsl,
                    op0=mybir.AluOpType.mult,
                    op1=mybir.AluOpType.mult,
                    accum_out=acc[:, cell:cell + 1],
                )

            abs_ = [load_ab(0), load_ab(1)]
            ws = [build_w(abs_[0]), build_w(abs_[1])]

            for l in range(lp):
                w = ws[l]
                for j in range(bn):
                    cell = l * bn + j
                    if l == lp - 1 and j == bn - 1:
                        # Final slab in two pieces so the very last STT (and
                        # the closing reduction) starts on a small transfer.
                        gt = gpool.tile([_PART, _TAIL1], fp32, tag="g")
                        nc.sync.dma_start(out=gt[:], in_=g[l, j][:, :_TAIL1])
                        stt(gt, w[:, :_TAIL1], cell)
                        gt2 = gpool.tile([_PART, _FREE - _TAIL1], fp32, tag="g")
                        nc.sync.dma_start(out=gt2[:], in_=g[l, j][:, _TAIL1:])
                        stt(gt2, w[:, _TAIL1:], _NCELL - 1)
                    else:
                        gt = gpool.tile([_PART, _FREE], fp32, tag="g")
                        nc.sync.dma_start(out=gt[:], in_=g[l, j])
                        stt(gt, w[:], cell)
                if l + 2 < lp:
                    abs_.append(load_ab(l + 2))
                    ws.append(build_w(abs_[l + 2]))

            # Reduce acc over the 128 partitions on-chip: ps[m] = sum_p acc[p, m]
            fps = psf.tile([_NCELL, 1], fp32)
            nc.tensor.matmul(fps[:], lhsT=acc[:], rhs=ones[:], start=True, stop=True)
            ft = small.tile([_NCELL, 1], fp32)
            nc.scalar.copy(out=ft[:], in_=fps[:])
            nc.sync.dma_start(out=out, in_=ft[:])

    nc.compile()
    return nc


_NC_CACHE = {}


def _get_module():
    if "nc" not in _NC_CACHE:
        _NC_CACHE["nc"] = build_module()
    return _NC_CACHE["nc"]


def make_in_maps(lora_A, lora_B, gradient):
    import ml_dtypes

    lora_A = np.asarray(lora_A, dtype=np.float32)
    lora_B = np.asarray(lora_B, dtype=np.float32)
    gradient = np.asarray(gradient, dtype=np.float32)
    in_maps = []
    for c in range(NCORES):
        sl = slice(LP * c, LP * (c + 1))
        # btp[l, r, c*128+p] = B[l, 8p+c, r]
        bt = lora_B[sl].transpose(0, 2, 1)
        btp = bt.reshape(LP, R, _PART, _RPP).transpose(0, 1, 3, 2).reshape(
            LP, R, OUT)
        ab = np.concatenate([btp, lora_A[sl]], axis=2)  # [LP, R, 2048]
        in_maps.append({
            "g": np.ascontiguousarray(
                gradient[sl].reshape(LP, BN, _PART, _FREE)),
            "ab": np.ascontiguousarray(ab.astype(ml_dtypes.bfloat16)),
        })
    return in_maps


def kernel(lora_A, lora_B, gradient, _trace=False, _trace_kwargs=None):
    from concourse.bass_utils import run_bass_kernel_spmd

    nc = _get_module()
    in_maps = make_in_maps(lora_A, lora_B, gradient)
    last_exc = None
    for attempt in range(3):
        try:
            res = run_bass_kernel_spmd(
                nc,
                in_maps,
                core_ids=list(range(NCORES)),
                trace=_trace,
                **(_trace_kwargs or {}),
            )
            break
        except Exception as e:  # transient device wedges (NRT_EXEC_UNIT_...)
            last_exc = e
            import time as _time

            _time.sleep(15 * (attempt + 1))
    else:
        raise last_exc
    total = np.zeros(BN, np.float64)
    for m in res.results:
        cells = m["out"].astype(np.float64).ravel()  # [LP*BN + 1]
        per_j = cells[:LP * BN].reshape(LP, BN).sum(axis=0)
        per_j[BN - 1] += cells[LP * BN]
        total += per_j
    out = total.astype(np.float32).reshape(B, N)
    if _trace:
        return out, res
    return out
